# revision 22
# baseline (speedup 1.0000x reference)
"""Multi-head attention (B=2, S=2048, D=768, H=12) on 8 trn2 NeuronCores.

Sharding: the 24 (batch, head) pairs are split 3-heads-per-core
(core c -> batch c//4, heads 3*(c%4) .. 3*(c%4)+2).  Each core computes
Q/K/V projections for its heads, attention, and a partial output
projection against its 192-column slice of w_o.  The host sums the 4
partial outputs per batch (the tensor-parallel all-reduce).

Device kernel layout notes:
 - activations are shipped pre-transposed (xT: [D, S]) so the contraction
   dim (D) lands on SBUF partitions with no on-device transpose
 - scores are computed transposed (S^T = K @ Q^T, [k, q]) so the
   attn @ V step needs no transpose of the softmax matrix; the softmax
   scale (1/8) is folded into the Q^T projection eviction
 - softmax denominators come for free from a ones-column appended to V
 - matmuls run in fp16 (fp32 PSUM accumulate); exp runs on the scalar
   engine from fp32 scores in [128, 1024] slabs
 - the scalar engine is the bottleneck (~1.1us per exp slab, 96 slabs),
   and Tile keeps per-engine program order, so the program is emitted as
   an explicitly woven pipeline: each exp "slot" carries the matching
   score matmuls, the previous slot's attn@V, and a filler unit (next
   head's projections / output projection) sized to the PE slack
"""

import sys

sys.path.insert(0, "/opt/trn_rl_repo")

import numpy as np

B, S, D = 2, 2048, 768
H, DK = 12, 64
HPC = 3          # heads per core
DP = HPC * DK    # 192: d' slice per core
NCORES = 8
DCH = D // 128   # 6 d-chunks
KC = S // 128    # 16 k-chunks
QB = 512         # q block (norm / AV / out-proj granularity)
NQB = S // QB    # 4
EB = 1024        # exp slab width (2 q blocks)
NEB = S // EB    # 2
SCALE = 1.0 / 8.0  # 1/sqrt(DK)

_compiled = {}
_DBG = False


def _build():
    import concourse.bass as bass
    import concourse.mybir as mybir
    import concourse.tile as tile
    from concourse import bacc
    from concourse.masks import make_identity

    f32 = mybir.dt.float32
    f16 = mybir.dt.float16

    nc = bacc.Bacc("TRN2", target_bir_lowering=False, debug=False)

    xqt = nc.dram_tensor("xqt", [D, S], f32, kind="ExternalInput")
    xkt = nc.dram_tensor("xkt", [D, S], f32, kind="ExternalInput")
    xvt = nc.dram_tensor("xvt", [D, S], f32, kind="ExternalInput")
    wqt = nc.dram_tensor("wqt", [D, DP], f32, kind="ExternalInput")
    wkt = nc.dram_tensor("wkt", [D, DP], f32, kind="ExternalInput")
    wvt = nc.dram_tensor("wvt", [D, DP], f32, kind="ExternalInput")
    wot = nc.dram_tensor("wot", [DP, D], f32, kind="ExternalInput")
    outt = nc.dram_tensor("outt", [D, S], f32, kind="ExternalOutput")
    dbg = nc.dram_tensor("dbg", [64, HPC, S], f16, kind="ExternalOutput") if _DBG else None

    with tile.TileContext(nc) as tc:
        with (
            tc.tile_pool(name="stage", bufs=3) as stage_pool,
            tc.tile_pool(name="resident", bufs=1) as res_pool,
            tc.tile_pool(name="heads", bufs=2) as head_pool,
            tc.tile_pool(name="pt", bufs=16) as pt_pool,
            tc.tile_pool(name="norm", bufs=2) as norm_pool,
            tc.tile_pool(name="ostage", bufs=1) as o_pool,
            tc.tile_pool(name="dram", bufs=3, space="DRAM") as dram_pool,
            tc.tile_pool(name="psS", bufs=2, space="PSUM") as psS,
            tc.tile_pool(name="psX", bufs=2, space="PSUM") as psX,
            tc.tile_pool(name="psP", bufs=2, space="PSUM") as psP,
        ):
            ring_state = [0]

            def next_ring():
                ring_state[0] ^= 1
                return nc.sync if ring_state[0] else nc.scalar

            # ---- weights: load f32, cast to f16 ----
            wq_bf = res_pool.tile([128, DCH, DP], f16, tag="wq_bf")
            wk_bf = res_pool.tile([128, DCH, DP], f16, tag="wk_bf")
            wv_bf = res_pool.tile([128, DCH, DP], f16, tag="wv_bf")
            for wdram, wbf in ((wqt, wq_bf), (wkt, wk_bf), (wvt, wv_bf)):
                for half in range(2):
                    wstg = stage_pool.tile([128, DCH // 2, DP], f32, tag="xstage")
                    sl = slice(half * (D // 2), (half + 1) * (D // 2))
                    next_ring().dma_start(
                        wstg[:], wdram[sl].rearrange("(c p) o -> p c o", p=128)
                    )
                    nc.vector.tensor_copy(
                        wbf[:, half * (DCH // 2) : (half + 1) * (DCH // 2), :],
                        wstg[:],
                    )
            wo_bf = res_pool.tile([64, HPC, D], f16, tag="wo_bf")
            for half in range(2):
                os_ = half * (D // 2)
                wo_stg = stage_pool.tile([64, HPC, D // 2], f32, tag="xstage")
                next_ring().dma_start(
                    wo_stg[:],
                    wot[:, os_ : os_ + D // 2].rearrange("(h p) o -> p h o", p=64),
                )
                nc.vector.tensor_copy(wo_bf[:, :, os_ : os_ + D // 2], wo_stg[:])

            ident = res_pool.tile([64, 64], f16, tag="ident")
            make_identity(nc, ident[:])

            qt_bf = res_pool.tile([128, DCH, S], f16, tag="qt_bf")
            kt_bf = res_pool.tile([128, DCH, S], f16, tag="kt_bf")
            vt_bf = res_pool.tile([128, DCH, S], f16, tag="vt_bf")

            def emit_load(xdram, xbf, i, ring=None):
                stg = stage_pool.tile([128, S], f32, tag="xstage")
                (ring or next_ring()).dma_start(
                    stg[:], xdram[128 * i : 128 * (i + 1), :]
                )
                nc.vector.tensor_copy(xbf[:, i, :], stg[:])

            # q,k loads first (gate the first scores); v behind
            for i in range(DCH):
                emit_load(xqt, qt_bf, i)
                emit_load(xkt, kt_bf, i)

            # XT: normalized attention outputs, transposed ([dk, q] per head)
            xt_all = res_pool.tile([64, HPC, S], f16, tag="xt_all")

            head_tiles = {}

            def alloc_head(h):
                head_tiles[h] = {
                    "qT": head_pool.tile([64, S], f16, tag="qT", name="qT"),
                    "kT": head_pool.tile([64, S], f16, tag="kT", name="kT"),
                    "vT": head_pool.tile([64, S], f16, tag="vT", name="vT"),
                    "vaug": head_pool.tile([128, KC, 65], f16, tag="vaug", name="vaug"),
                }

            def proj_halves(h, kind, j):
                """A 512-col projection block as two filler halves of 3
                matmuls each (sized to the PE slack of one exp slot)."""
                wbf, xbf, scale = {
                    "qT": (wq_bf, qt_bf, SCALE),
                    "kT": (wk_bf, kt_bf, None),
                    "vT": (wv_bf, vt_bf, None),
                }[kind]
                dst = head_tiles[h][kind]
                state = {}

                def emit_half(part):
                    if part == 0:
                        state["pp"] = psP.tile([64, QB], f32, tag="pp", name="pp")
                    pp = state["pp"]
                    for i in range(3 * part, 3 * part + 3):
                        nc.tensor.matmul(
                            pp[:],
                            wbf[:, i, 64 * h : 64 * (h + 1)],
                            xbf[:, i, QB * j : QB * (j + 1)],
                            start=(i == 0),
                            stop=(i == DCH - 1),
                        )
                    if part == 1:
                        dst_s = dst[:, QB * j : QB * (j + 1)]
                        if scale is None:
                            nc.vector.tensor_copy(dst_s, pp[:])
                        else:
                            nc.vector.tensor_scalar_mul(dst_s, pp[:], scale)

                return [lambda: emit_half(0), lambda: emit_half(1)]

            def proj_unit(h, kind, j):
                halves = proj_halves(h, kind, j)

                def emit():
                    halves[0]()
                    halves[1]()

                return emit

            def vaug_ones_unit(h):
                def emit():
                    nc.vector.memset(head_tiles[h]["vaug"][:, :, 64:65], 1.0)

                return emit

            def vaug_unit(h, t4):
                """4 PE transposes of vT -> vaug rows, one evict."""

                def emit():
                    vT = head_tiles[h]["vT"]
                    vaug = head_tiles[h]["vaug"]
                    tp = psP.tile([128, 4, 64], f16, tag="pp")
                    for u in range(4):
                        t = 4 * t4 + u
                        nc.tensor.transpose(
                            tp[:, u, :], vT[:, 128 * t : 128 * (t + 1)], ident[:]
                        )
                    nc.vector.tensor_copy(vaug[:, 4 * t4 : 4 * t4 + 4, 0:64], tp[:])

                return emit

            def head_units(h, with_v=True):
                units = []
                for j in range(NQB):
                    units.extend(proj_halves(h, "kT", j))
                for j in range(NQB):
                    units.extend(proj_halves(h, "qT", j))
                if with_v:
                    units.append(vaug_ones_unit(h))
                    for j in range(NQB):
                        units.extend(proj_halves(h, "vT", j))
                    for t4 in range(KC // 4):
                        units.append(vaug_unit(h, t4))
                return units

            def outproj_unit(j, m):
                def emit():
                    op = psP.tile([128, QB], f32, tag="pp")
                    for h in range(HPC):
                        nc.tensor.matmul(
                            op[:],
                            wo_bf[:, h, 128 * m : 128 * (m + 1)],
                            xt_all[:, h, QB * j : QB * (j + 1)],
                            start=(h == 0),
                            stop=(h == HPC - 1),
                        )
                    ob = o_pool.tile([128, QB], f32, tag="ob")
                    nc.vector.tensor_copy(ob[:], op[:])
                    nc.sync.dma_start(
                        outt[128 * m : 128 * (m + 1), QB * j : QB * (j + 1)],
                        ob[:],
                    )

                return emit

            def norm_unit(h, j, xp):
                def emit():
                    xaug = norm_pool.tile([65, QB], f32, tag="xaug")
                    nc.vector.tensor_copy(xaug[:], xp[:])
                    den_b = norm_pool.tile([64, QB], f32, tag="den_b")
                    scratch = dram_pool.tile([1, QB], f32, tag="denrow")
                    nc.sync.dma_start(scratch[:], xaug[64:65, :])
                    srow = scratch[:]
                    nc.sync.dma_start(
                        den_b[:],
                        bass.AP(
                            tensor=srow.tensor,
                            offset=srow.offset,
                            ap=[[0, 64]] + [list(a) for a in srow.ap[1:]],
                        ),
                    )
                    nc.vector.reciprocal_approx_fast(den_b[:], den_b[:])
                    nc.vector.tensor_tensor(
                        xt_all[:, h, QB * j : QB * (j + 1)],
                        xaug[0:64, :],
                        den_b[:],
                        mybir.AluOpType.mult,
                    )

                return emit

            # ---- prologue: head-0 q/k projections, then v loads, v proj ----
            alloc_head(0)
            for u in head_units(0, with_v=False):
                u()
            for i in range(DCH):
                emit_load(xvt, vt_bf, i, ring=nc.sync)

            def head0_v_units():
                # emitted after slab-0's scores so the PE stream never
                # stalls on the v DMAs ahead of the first exps
                nc.vector.memset(head_tiles[0]["vaug"][:, :, 64:65], 1.0)
                for j in range(NQB):
                    proj_unit(0, "vT", j)()
                for t4 in range(KC // 4):
                    vaug_unit(0, t4)()

            # ---- woven attention pipeline ----
            fillers = []        # queue of PE filler units
            pending = []        # ops to emit at slot 0 of the next slab
            slabs = [(h, e) for h in range(HPC) for e in range(NEB)]
            for si, (h, e) in enumerate(slabs):
                qT = head_tiles[h]["qT"]
                kT = head_tiles[h]["kT"]
                vaug = head_tiles[h]["vaug"]
                if e == 0 and h + 1 < HPC:
                    alloc_head(h + 1)
                    fillers.extend(head_units(h + 1))
                if h == HPC - 1 and e == 1:
                    # out-proj for the first slab's blocks (normed by now)
                    for j in (0, 1):
                        for m in range(DCH):
                            fillers.append(outproj_unit(j, m))
                hold = {}

                def get_xp(hold=hold):
                    # allocated lazily so the previous slab's deferred norm
                    # reads are emitted before these slots are recycled
                    if "a" not in hold:
                        hold["a"] = psX.tile([65, QB], f32, tag="xp", name="xpa")
                        hold["b"] = psX.tile([65, QB], f32, tag="xp", name="xpb")
                    return hold["a"], hold["b"]

                def emit_av(t, vaug=vaug, get_xp=get_xp, pts=None):
                    xpa, xpb = get_xp()
                    pt = pts[t]
                    nc.tensor.matmul(
                        xpa[:], vaug[:, t, :], pt[:, 0:QB],
                        start=(t == 0), stop=(t == KC - 1),
                    )
                    nc.tensor.matmul(
                        xpb[:], vaug[:, t, :], pt[:, QB:EB],
                        start=(t == 0), stop=(t == KC - 1),
                    )

                pts = []
                for t in range(KC):
                    sp = psS.tile([128, EB], f32, tag="sp")
                    for half in range(2):
                        nc.tensor.matmul(
                            sp[:, QB * half : QB * (half + 1)],
                            kT[:, 128 * t : 128 * (t + 1)],
                            qT[:, EB * e + QB * half : EB * e + QB * (half + 1)],
                            start=True,
                            stop=True,
                        )
                    pt = pt_pool.tile([128, EB], f16, tag="pt")
                    pts.append(pt)
                    nc.scalar.activation(
                        pt[:], sp[:], mybir.ActivationFunctionType.Exp
                    )
                    if t == 0:
                        for op in pending:
                            op()
                        pending = []
                    else:
                        if si > 0:
                            emit_av(t - 1, pts=pts)
                        if fillers:
                            fillers.pop(0)()
                if si == 0:
                    head0_v_units()
                    for t in range(KC - 1):
                        emit_av(t, pts=pts)
                xpa, xpb = get_xp()
                last_av = lambda pts=pts, emit_av=emit_av: emit_av(KC - 1, pts=pts)
                pending = [
                    last_av,
                    norm_unit(h, 2 * e, xpa),
                    norm_unit(h, 2 * e + 1, xpb),
                ]

            # ---- tail: final AV + norms + remaining output projection ----
            for op in pending:
                op()
            if _DBG:
                nc.sync.dma_start(dbg[:], xt_all[:])
            for u in fillers:
                u()
            for j in (2, 3):
                for m in range(DCH):
                    outproj_unit(j, m)()

    nc.compile()
    return nc


def _get_nc():
    if "nc" not in _compiled:
        _compiled["nc"] = _build()
    return _compiled["nc"]


def _shard(q, k, v, w_q, w_k, w_v, w_o):
    in_maps = []
    for c in range(NCORES):
        b, g = divmod(c, NCORES // B)
        cols = slice(DP * g, DP * (g + 1))
        in_maps.append(
            {
                "xqt": np.ascontiguousarray(q[b].T),
                "xkt": np.ascontiguousarray(k[b].T),
                "xvt": np.ascontiguousarray(v[b].T),
                "wqt": np.ascontiguousarray(w_q[cols, :].T),
                "wkt": np.ascontiguousarray(w_k[cols, :].T),
                "wvt": np.ascontiguousarray(w_v[cols, :].T),
                "wot": np.ascontiguousarray(w_o[:, cols].T),
            }
        )
    return in_maps


def kernel(q, k, v, w_q, w_k, w_v, w_o, _trace=False):
    from concourse.bass_utils import run_bass_kernel_spmd

    q = np.asarray(q, np.float32)
    k = np.asarray(k, np.float32)
    v = np.asarray(v, np.float32)
    w_q = np.asarray(w_q, np.float32)
    w_k = np.asarray(w_k, np.float32)
    w_v = np.asarray(w_v, np.float32)
    w_o = np.asarray(w_o, np.float32)

    nc = _get_nc()
    in_maps = _shard(q, k, v, w_q, w_k, w_v, w_o)
    res = run_bass_kernel_spmd(
        nc, in_maps, list(range(NCORES)), trace=_trace
    )
    out = np.zeros((B, S, D), np.float32)
    for c in range(NCORES):
        b = c // (NCORES // B)
        out[b] += res.results[c]["outt"].T
    if _trace:
        return out, res
    return out


# revision 23
# speedup vs baseline: 1.2036x; 1.2036x over previous
"""Multi-head attention (B=2, S=2048, D=768, H=12) on 8 trn2 NeuronCores.

Sharding: the 24 (batch, head) pairs are split 3-heads-per-core
(core c -> batch c//4, heads 3*(c%4) .. 3*(c%4)+2).  Each core computes
Q/K/V projections for its heads, attention, and a partial output
projection against its 192-column slice of w_o.  The host sums the 4
partial outputs per batch (the tensor-parallel all-reduce).

Device kernel layout notes:
 - activations are shipped pre-transposed (xT: [D, S]) so the contraction
   dim (D) lands on SBUF partitions with no on-device transpose
 - scores are computed transposed (S^T = K @ Q^T, [k, q]) so the
   attn @ V step needs no transpose of the softmax matrix; the softmax
   scale (1/8) is folded into the Q^T projection eviction
 - softmax denominators come for free from a ones-column appended to V
 - matmuls run in fp16 (fp32 PSUM accumulate); exp runs on the scalar
   engine from fp32 scores in [128, 1024] slabs
 - the scalar engine is the bottleneck (~1.1us per exp slab, 96 slabs),
   and Tile keeps per-engine program order, so the program is emitted as
   an explicitly woven pipeline: each exp "slot" carries the matching
   score matmuls, the previous slot's attn@V, and a filler unit (next
   head's projections / output projection) sized to the PE slack
"""

import sys

sys.path.insert(0, "/opt/trn_rl_repo")

import numpy as np

B, S, D = 2, 2048, 768
H, DK = 12, 64
HPC = 3          # heads per core
DP = HPC * DK    # 192: d' slice per core
NCORES = 8
DCH = D // 128   # 6 d-chunks
KC = S // 128    # 16 k-chunks
QB = 512         # q block (norm / AV / out-proj granularity)
NQB = S // QB    # 4
EB = 1024        # exp slab width (2 q blocks)
NEB = S // EB    # 2
SCALE = 1.0 / 8.0  # 1/sqrt(DK)

_compiled = {}
_DBG = False


def _build():
    import concourse.bass as bass
    import concourse.mybir as mybir
    import concourse.tile as tile
    from concourse import bacc
    from concourse.masks import make_identity

    f32 = mybir.dt.float32
    f16 = mybir.dt.float16

    nc = bacc.Bacc("TRN2", target_bir_lowering=False, debug=False)

    xqt = nc.dram_tensor("xqt", [D, S], f32, kind="ExternalInput")
    xkt = nc.dram_tensor("xkt", [D, S], f32, kind="ExternalInput")
    xvt = nc.dram_tensor("xvt", [D, S], f32, kind="ExternalInput")
    wqt = nc.dram_tensor("wqt", [D, DP], f32, kind="ExternalInput")
    wkt = nc.dram_tensor("wkt", [D, DP], f32, kind="ExternalInput")
    wvt = nc.dram_tensor("wvt", [D, DP], f32, kind="ExternalInput")
    wot = nc.dram_tensor("wot", [DP, D], f32, kind="ExternalInput")
    outt = nc.dram_tensor("outt", [D, S], f32, kind="ExternalOutput")
    dbg = nc.dram_tensor("dbg", [64, HPC, S], f16, kind="ExternalOutput") if _DBG else None

    with tile.TileContext(nc) as tc:
        with (
            tc.tile_pool(name="stage", bufs=5) as stage_pool,
            tc.tile_pool(name="resident", bufs=1) as res_pool,
            tc.tile_pool(name="heads", bufs=2) as head_pool,
            tc.tile_pool(name="pt", bufs=16) as pt_pool,
            tc.tile_pool(name="norm", bufs=2) as norm_pool,
            tc.tile_pool(name="ostage", bufs=2) as o_pool,
            tc.tile_pool(name="dram", bufs=3, space="DRAM") as dram_pool,
            tc.tile_pool(name="psS", bufs=2, space="PSUM") as psS,
            tc.tile_pool(name="psX", bufs=2, space="PSUM") as psX,
            tc.tile_pool(name="psP", bufs=2, space="PSUM") as psP,
        ):
            ring_state = [0]

            def next_ring():
                ring_state[0] ^= 1
                return nc.sync if ring_state[0] else nc.scalar

            # ---- weights: load f32, cast to f16 ----
            wq_bf = res_pool.tile([128, DCH, DP], f16, tag="wq_bf")
            wk_bf = res_pool.tile([128, DCH, DP], f16, tag="wk_bf")
            wv_bf = res_pool.tile([128, DCH, DP], f16, tag="wv_bf")
            for wdram, wbf in ((wqt, wq_bf), (wkt, wk_bf), (wvt, wv_bf)):
                for half in range(2):
                    wstg = stage_pool.tile([128, DCH // 2, DP], f32, tag="xstage")
                    sl = slice(half * (D // 2), (half + 1) * (D // 2))
                    next_ring().dma_start(
                        wstg[:], wdram[sl].rearrange("(c p) o -> p c o", p=128)
                    )
                    nc.vector.tensor_copy(
                        wbf[:, half * (DCH // 2) : (half + 1) * (DCH // 2), :],
                        wstg[:],
                    )
            wo_bf = res_pool.tile([64, HPC, D], f16, tag="wo_bf")
            for half in range(2):
                os_ = half * (D // 2)
                wo_stg = stage_pool.tile([64, HPC, D // 2], f32, tag="xstage")
                next_ring().dma_start(
                    wo_stg[:],
                    wot[:, os_ : os_ + D // 2].rearrange("(h p) o -> p h o", p=64),
                )
                nc.vector.tensor_copy(wo_bf[:, :, os_ : os_ + D // 2], wo_stg[:])

            ident = res_pool.tile([64, 64], f16, tag="ident")
            make_identity(nc, ident[:])

            qt_bf = res_pool.tile([128, DCH, S], f16, tag="qt_bf")
            kt_bf = res_pool.tile([128, DCH, S], f16, tag="kt_bf")
            vt_bf = res_pool.tile([128, DCH, S], f16, tag="vt_bf")

            def emit_load(xdram, xbf, i, ring=None):
                for half in range(2):
                    cs = half * (S // 2)
                    stg = stage_pool.tile([128, S // 2], f32, tag="xstage")
                    (ring or next_ring()).dma_start(
                        stg[:], xdram[128 * i : 128 * (i + 1), cs : cs + S // 2]
                    )
                    nc.vector.tensor_copy(xbf[:, i, cs : cs + S // 2], stg[:])

            # q,k loads first (gate the first scores); v behind
            for i in range(DCH):
                emit_load(xqt, qt_bf, i)
                emit_load(xkt, kt_bf, i)

            # XT: normalized attention outputs, transposed ([dk, q] per head)
            xt_all = res_pool.tile([64, HPC, S], f16, tag="xt_all")

            head_tiles = {}

            def alloc_head(h):
                head_tiles[h] = {
                    "qT": head_pool.tile([64, S], f16, tag="qT", name="qT"),
                    "kT": head_pool.tile([64, S], f16, tag="kT", name="kT"),
                    "vT": head_pool.tile([64, S], f16, tag="vT", name="vT"),
                    "vaug": head_pool.tile([128, KC, 65], f16, tag="vaug", name="vaug"),
                }

            def proj_halves(h, kind, j):
                """A 512-col projection block as two filler halves of 3
                matmuls each (sized to the PE slack of one exp slot)."""
                wbf, xbf, scale = {
                    "qT": (wq_bf, qt_bf, SCALE),
                    "kT": (wk_bf, kt_bf, None),
                    "vT": (wv_bf, vt_bf, None),
                }[kind]
                dst = head_tiles[h][kind]
                state = {}

                def emit_half(part):
                    if part == 0:
                        state["pp"] = psP.tile([64, QB], f32, tag="pp", name="pp")
                    pp = state["pp"]
                    for i in range(3 * part, 3 * part + 3):
                        nc.tensor.matmul(
                            pp[:],
                            wbf[:, i, 64 * h : 64 * (h + 1)],
                            xbf[:, i, QB * j : QB * (j + 1)],
                            start=(i == 0),
                            stop=(i == DCH - 1),
                        )
                    if part == 1:
                        dst_s = dst[:, QB * j : QB * (j + 1)]
                        if scale is None:
                            nc.vector.tensor_copy(dst_s, pp[:])
                        else:
                            nc.vector.tensor_scalar_mul(dst_s, pp[:], scale)

                return [lambda: emit_half(0), lambda: emit_half(1)]

            def proj_unit(h, kind, j):
                halves = proj_halves(h, kind, j)

                def emit():
                    halves[0]()
                    halves[1]()

                return emit

            def vaug_ones_unit(h):
                def emit():
                    nc.vector.memset(head_tiles[h]["vaug"][:, :, 64:65], 1.0)

                return emit

            def vaug_unit(h, t4):
                """4 PE transposes of vT -> vaug rows, one evict."""

                def emit():
                    vT = head_tiles[h]["vT"]
                    vaug = head_tiles[h]["vaug"]
                    tp = psP.tile([128, 4, 64], f16, tag="pp")
                    for u in range(4):
                        t = 4 * t4 + u
                        nc.tensor.transpose(
                            tp[:, u, :], vT[:, 128 * t : 128 * (t + 1)], ident[:]
                        )
                    nc.vector.tensor_copy(vaug[:, 4 * t4 : 4 * t4 + 4, 0:64], tp[:])

                return emit

            def head_units(h, with_v=True):
                units = []
                for j in range(NQB):
                    units.extend(proj_halves(h, "kT", j))
                for j in range(NQB):
                    units.extend(proj_halves(h, "qT", j))
                if with_v:
                    units.append(vaug_ones_unit(h))
                    for j in range(NQB):
                        units.extend(proj_halves(h, "vT", j))
                    for t4 in range(KC // 4):
                        units.append(vaug_unit(h, t4))
                return units

            def outproj_unit(j, m):
                def emit():
                    op = psP.tile([128, QB], f32, tag="pp")
                    for h in range(HPC):
                        nc.tensor.matmul(
                            op[:],
                            wo_bf[:, h, 128 * m : 128 * (m + 1)],
                            xt_all[:, h, QB * j : QB * (j + 1)],
                            start=(h == 0),
                            stop=(h == HPC - 1),
                        )
                    ob = o_pool.tile([128, QB], f32, tag="ob")
                    nc.vector.tensor_copy(ob[:], op[:])
                    nc.sync.dma_start(
                        outt[128 * m : 128 * (m + 1), QB * j : QB * (j + 1)],
                        ob[:],
                    )

                return emit

            def norm_unit(h, j, xp):
                def emit():
                    xaug = norm_pool.tile([65, QB], f32, tag="xaug")
                    nc.vector.tensor_copy(xaug[:], xp[:])
                    den_b = norm_pool.tile([64, QB], f32, tag="den_b")
                    scratch = dram_pool.tile([1, QB], f32, tag="denrow")
                    nc.sync.dma_start(scratch[:], xaug[64:65, :])
                    srow = scratch[:]
                    nc.sync.dma_start(
                        den_b[:],
                        bass.AP(
                            tensor=srow.tensor,
                            offset=srow.offset,
                            ap=[[0, 64]] + [list(a) for a in srow.ap[1:]],
                        ),
                    )
                    nc.vector.reciprocal_approx_fast(den_b[:], den_b[:])
                    nc.vector.tensor_tensor(
                        xt_all[:, h, QB * j : QB * (j + 1)],
                        xaug[0:64, :],
                        den_b[:],
                        mybir.AluOpType.mult,
                    )

                return emit

            # ---- prologue: head-0 q/k projections, then v loads, v proj ----
            alloc_head(0)
            for u in head_units(0, with_v=False):
                u()
            for i in range(DCH):
                emit_load(xvt, vt_bf, i, ring=nc.sync)

            def head0_v_units():
                # emitted after slab-0's scores so the PE stream never
                # stalls on the v DMAs ahead of the first exps
                nc.vector.memset(head_tiles[0]["vaug"][:, :, 64:65], 1.0)
                for j in range(NQB):
                    proj_unit(0, "vT", j)()
                for t4 in range(KC // 4):
                    vaug_unit(0, t4)()

            # ---- woven attention pipeline ----
            fillers = []        # queue of PE filler units
            pending = []        # ops to emit at slot 0 of the next slab
            slabs = [(h, e) for h in range(HPC) for e in range(NEB)]
            for si, (h, e) in enumerate(slabs):
                qT = head_tiles[h]["qT"]
                kT = head_tiles[h]["kT"]
                vaug = head_tiles[h]["vaug"]
                if e == 0 and h + 1 < HPC:
                    alloc_head(h + 1)
                    fillers.extend(head_units(h + 1))
                if h == HPC - 1 and e == 1:
                    # out-proj for the first slab's blocks (normed by now)
                    for j in (0, 1):
                        for m in range(DCH):
                            fillers.append(outproj_unit(j, m))
                hold = {}

                def get_xp(hold=hold):
                    # allocated lazily so the previous slab's deferred norm
                    # reads are emitted before these slots are recycled
                    if "a" not in hold:
                        hold["a"] = psX.tile([65, QB], f32, tag="xp", name="xpa")
                        hold["b"] = psX.tile([65, QB], f32, tag="xp", name="xpb")
                    return hold["a"], hold["b"]

                def emit_av(t, vaug=vaug, get_xp=get_xp, pts=None):
                    xpa, xpb = get_xp()
                    pt = pts[t]
                    nc.tensor.matmul(
                        xpa[:], vaug[:, t, :], pt[:, 0:QB],
                        start=(t == 0), stop=(t == KC - 1),
                    )
                    nc.tensor.matmul(
                        xpb[:], vaug[:, t, :], pt[:, QB:EB],
                        start=(t == 0), stop=(t == KC - 1),
                    )

                pts = []
                for t in range(KC):
                    sp = psS.tile([128, EB], f32, tag="sp")
                    for half in range(2):
                        nc.tensor.matmul(
                            sp[:, QB * half : QB * (half + 1)],
                            kT[:, 128 * t : 128 * (t + 1)],
                            qT[:, EB * e + QB * half : EB * e + QB * (half + 1)],
                            start=True,
                            stop=True,
                        )
                    pt = pt_pool.tile([128, EB], f16, tag="pt")
                    pts.append(pt)
                    nc.scalar.activation(
                        pt[:], sp[:], mybir.ActivationFunctionType.Exp
                    )
                    if t == 0:
                        for op in pending:
                            op()
                        pending = []
                    else:
                        if si > 0:
                            emit_av(t - 1, pts=pts)
                        if fillers:
                            fillers.pop(0)()
                if si == 0:
                    head0_v_units()
                    for t in range(KC - 1):
                        emit_av(t, pts=pts)
                xpa, xpb = get_xp()
                last_av = lambda pts=pts, emit_av=emit_av: emit_av(KC - 1, pts=pts)
                pending = [
                    last_av,
                    norm_unit(h, 2 * e, xpa),
                    norm_unit(h, 2 * e + 1, xpb),
                ]

            # ---- tail: final AV + norms + remaining output projection ----
            for op in pending:
                op()
            if _DBG:
                nc.sync.dma_start(dbg[:], xt_all[:])
            for u in fillers:
                u()
            for j in (2, 3):
                for m in range(DCH):
                    outproj_unit(j, m)()

    nc.compile()
    return nc


def _get_nc():
    if "nc" not in _compiled:
        _compiled["nc"] = _build()
    return _compiled["nc"]


def _shard(q, k, v, w_q, w_k, w_v, w_o):
    in_maps = []
    for c in range(NCORES):
        b, g = divmod(c, NCORES // B)
        cols = slice(DP * g, DP * (g + 1))
        in_maps.append(
            {
                "xqt": np.ascontiguousarray(q[b].T),
                "xkt": np.ascontiguousarray(k[b].T),
                "xvt": np.ascontiguousarray(v[b].T),
                "wqt": np.ascontiguousarray(w_q[cols, :].T),
                "wkt": np.ascontiguousarray(w_k[cols, :].T),
                "wvt": np.ascontiguousarray(w_v[cols, :].T),
                "wot": np.ascontiguousarray(w_o[:, cols].T),
            }
        )
    return in_maps


def kernel(q, k, v, w_q, w_k, w_v, w_o, _trace=False):
    from concourse.bass_utils import run_bass_kernel_spmd

    q = np.asarray(q, np.float32)
    k = np.asarray(k, np.float32)
    v = np.asarray(v, np.float32)
    w_q = np.asarray(w_q, np.float32)
    w_k = np.asarray(w_k, np.float32)
    w_v = np.asarray(w_v, np.float32)
    w_o = np.asarray(w_o, np.float32)

    nc = _get_nc()
    in_maps = _shard(q, k, v, w_q, w_k, w_v, w_o)
    res = run_bass_kernel_spmd(
        nc, in_maps, list(range(NCORES)), trace=_trace
    )
    out = np.zeros((B, S, D), np.float32)
    for c in range(NCORES):
        b = c // (NCORES // B)
        out[b] += res.results[c]["outt"].T
    if _trace:
        return out, res
    return out
